# revision 1
# baseline (speedup 1.0000x reference)
"""Trainium2 Bass kernel for nn_Attention_35734127903400.

Dense transformer attention block:
  xq = LN(x@wq); xk = LN(x@wk); xv = x@wv          (LN over full flattened head dim)
  rope(q, k); GQA self-attention (16 q heads, 8 kv heads, S=2048, full/non-causal)
  gated cross-attention with y (128 tokens); out = (self + tanh(gate)*cross) @ wo

Sharding (8 cores, no collectives): token-sharded. Core c handles batch
b=c//2, sequence half hf=c%2 (1024 q tokens). Each core computes K/V for
its batch's FULL 2048-token sequence (replicated within the pair; +19%
proj FLOPs but zero communication), Q only for its local 1024 tokens.
LN is over the feature dim so it is fully core-local under this sharding.

Numerics: bf16 matmul operands, f32 PSUM accumulation, LN/softmax math in
f32. Softmax skips max-subtraction (q/k are LN'd so |score*scale| stays
far inside exp's f32 range).

Structure: projection stages build Q^T/K^T (LN+rope, PE-transposed to
head-major) and V fully in SBUF, spill each with ONE DMA to DRAM scratch
and free the SBUF (the single-writer/single-reader round trip keeps every
DMA's wait list within the 2-command HWDGE limit, and lets stage-local
pools stack LIFO). The attention stage reloads all three, then per
(head, q-chunk): S^T = K Q^T per 128-key chunk, exp on ACT into bf16,
softmax denominator via ones-matmul (every PSUM partition gets the column
sum), O^T = V^T E so merged heads land feature-major — exactly the rhs
layout the output projection needs. Output is out^T, transposed on host.
"""

import numpy as np
import ml_dtypes

import concourse.bass as bass
import concourse.mybir as mybir
import concourse.tile as tile
from concourse.bass_utils import run_bass_kernel_spmd
from concourse.masks import make_identity

BF16 = ml_dtypes.bfloat16
F32 = mybir.dt.float32
BF = mybir.dt.bfloat16

P = 128
B, S, D = 4, 2048, 2048
H, KVH = 16, 8
HD = 128
NREP = 2
YL, YD = 128, 1024
EPS = 1e-5
S_LOC = S // 2
DC = D // P          # 16 contraction chunks for D
YDC = YD // P        # 8
TC = S // P          # 16 token chunks (full seq)
TCL = S_LOC // P     # 8 local token chunks
NQ = 512             # q-free chunk (one PSUM bank of f32)
QCN = S_LOC // NQ    # 2
KVD = KVH * HD       # 1024
SCALE = 1.0 / float(np.sqrt(np.float32(HD)))
AF = mybir.ActivationFunctionType
ALU = mybir.AluOpType

_CACHED = {}
LAST_EXEC_NS = None


def _ln_stats(nc, statp, ps_chunks):
    """bn_stats over a list of [P, 512] psum chunks -> mv [P, 2] (mean, var)."""
    nchunks = len(ps_chunks)
    stats = statp.tile([P, nchunks, 6], F32, tag="bnstats")
    for i, ps in enumerate(ps_chunks):
        nc.vector.bn_stats(out=stats[:, i, :], in_=ps[:])
    mv = statp.tile([P, 2], F32, tag="bnaggr")
    nc.vector.bn_aggr(out=mv, in_=stats)
    return mv


def _rope_inplace(nc, ropep, zn, nheads, cos_t, sin_t):
    """In-place rope on zn [P, nheads*HD] f32; cos/sin [P, 64] f32."""
    zv = zn.rearrange("p (h f two) -> p h f two", h=nheads, two=2)
    re = zv[:, :, :, 0]
    im = zv[:, :, :, 1]
    shp = (P, nheads, HD // 2)
    cb = cos_t[:, None, :].to_broadcast(shp)
    sb = sin_t[:, None, :].to_broadcast(shp)
    t1 = ropep.tile([P, nheads, HD // 2], F32, tag="rp1")
    t2 = ropep.tile([P, nheads, HD // 2], F32, tag="rp2")
    t3 = ropep.tile([P, nheads, HD // 2], F32, tag="rp3")
    nc.vector.tensor_mul(out=t1, in0=re, in1=cb)   # re*c
    nc.vector.tensor_mul(out=t2, in0=re, in1=sb)   # re*s
    nc.vector.tensor_mul(out=t3, in0=im, in1=sb)   # im*s
    nc.vector.tensor_sub(out=re, in0=t1, in1=t3)   # re' = re*c - im*s
    nc.vector.tensor_mul(out=t3, in0=im, in1=cb)   # im*c
    nc.vector.tensor_add(out=im, in0=t2, in1=t3)   # im' = re*s + im*c


def _split_dma_waits(nc, max_waits=1):
    """This walrus build's per-instruction structs have very few embedded
    sync-wait slots (1-2 depending on opcode). Hoist excess waits of ANY
    instruction onto preceding same-engine single-wait NoOps — the sequencer
    executes them in stream order before the instruction, so semantics are
    identical (marginally more conservative)."""
    n_split = 0
    for f in nc.m.functions:
        for blk in f.blocks:
            insts = list(blk.instructions)
            out = []
            changed = False
            for ins in insts:
                si = ins.sync_info
                if (si is not None and si.on_wait
                        and len(si.on_wait) > max_waits):
                    waits = list(si.on_wait)
                    for wi, w in enumerate(waits[:-max_waits]):
                        out.append(mybir.InstNoOp(
                            name=f"{ins.name}-wsplit{wi}", engine=ins.engine,
                            sync_info=mybir.SyncInfo(on_wait=[w],
                                                     on_update=[])))
                    ins.sync_info = mybir.SyncInfo(
                        on_wait=waits[-max_waits:],
                        on_update=list(si.on_update))
                    changed = True
                    n_split += 1
                out.append(ins)
            if changed:
                blk.instructions = out
    return n_split


def build_program():
    nc = bass.Bass()

    # ---- I/O ----
    xT = nc.declare_dram_parameter("xT", [D, S], BF, isOutput=False)
    xTq = nc.declare_dram_parameter("xTq", [D, S_LOC], BF, isOutput=False)
    yT = nc.declare_dram_parameter("yT", [YD, YL], BF, isOutput=False)
    wq_d = nc.declare_dram_parameter("wq", [D, D], BF, isOutput=False)
    wkv_d = nc.declare_dram_parameter("wkv", [D, 2 * KVD], BF, isOutput=False)
    wkvy_d = nc.declare_dram_parameter("wkvy", [YD, 2 * KVD], BF, isOutput=False)
    wo_d = nc.declare_dram_parameter("wo", [D, D], BF, isOutput=False)
    qw_d = nc.declare_dram_parameter("qw", [D], F32, isOutput=False)
    qb_d = nc.declare_dram_parameter("qb", [D], F32, isOutput=False)
    kw_d = nc.declare_dram_parameter("kw", [KVD], F32, isOutput=False)
    kb_d = nc.declare_dram_parameter("kb", [KVD], F32, isOutput=False)
    kyw_d = nc.declare_dram_parameter("kyw", [KVD], F32, isOutput=False)
    kyb_d = nc.declare_dram_parameter("kyb", [KVD], F32, isOutput=False)
    cosq_d = nc.declare_dram_parameter("cosq", [S_LOC, HD // 2], F32, isOutput=False)
    sinq_d = nc.declare_dram_parameter("sinq", [S_LOC, HD // 2], F32, isOutput=False)
    cosk_d = nc.declare_dram_parameter("cosk", [S, HD // 2], F32, isOutput=False)
    sink_d = nc.declare_dram_parameter("sink", [S, HD // 2], F32, isOutput=False)
    gates_d = nc.declare_dram_parameter("gates", [H], F32, isOutput=False)
    ymb_d = nc.declare_dram_parameter("ymb", [YL], F32, isOutput=False)
    outT = nc.declare_dram_parameter("outT", [D, S_LOC], F32, isOutput=True)

    with tile.TileContext(nc) as tc:
        from contextlib import ExitStack
        with ExitStack() as ctx:
            # ---- persistent pools ----
            cpool = ctx.enter_context(tc.tile_pool(name="consts", bufs=1))
            yp = ctx.enter_context(tc.tile_pool(name="ypool", bufs=1))
            dramp = ctx.enter_context(
                tc.tile_pool(name="dscratch", bufs=1, space="DRAM"))
            QT_dram = dramp.tile([P, H, S_LOC], BF)
            KT_dram = dramp.tile([P, KVH, S], BF)
            V_dram = dramp.tile([P, TC, KVD], BF)

            # projection-phase transient pools
            lnp = tc.alloc_tile_pool(name="lnparams", bufs=1)
            xs = tc.alloc_tile_pool(name="xstream", bufs=3)
            work = tc.alloc_tile_pool(name="work", bufs=3)
            ropep = tc.alloc_tile_pool(name="rope", bufs=1)
            statp = tc.alloc_tile_pool(name="stats", bufs=3)
            psA = tc.alloc_tile_pool(name="psA", bufs=1, space="PSUM")

            # ---- constants ----
            ident = cpool.tile([P, P], F32)
            make_identity(nc, ident)
            ones_t = cpool.tile([P, P], BF)
            nc.vector.memset(ones_t, 1.0)
            eps_t = cpool.tile([P, 1], F32)
            nc.vector.memset(eps_t, EPS)
            gates_t = cpool.tile([P, H], F32)
            nc.gpsimd.dma_start(
                out=gates_t,
                in_=bass.AP(tensor=gates_d, offset=0, ap=[[0, P], [1, H]]))
            ymb_t = cpool.tile([P, 1], F32)
            nc.gpsimd.dma_start(
                out=ymb_t,
                in_=bass.AP(tensor=ymb_d, offset=0, ap=[[1, P], [0, 1]]))

            def bcast_vec(dram_h, n):
                t = lnp.tile([P, n], F32, tag=f"lnp_{dram_h.name}", bufs=1)
                nc.gpsimd.dma_start(
                    out=t, in_=bass.AP(tensor=dram_h, offset=0, ap=[[0, P], [1, n]]))
                return t

            qw_t = bcast_vec(qw_d, D)
            qb_t = bcast_vec(qb_d, D)
            kw_t = bcast_vec(kw_d, KVD)
            kb_t = bcast_vec(kb_d, KVD)
            kyw_t = bcast_vec(kyw_d, KVD)
            kyb_t = bcast_vec(kyb_d, KVD)

            YKT = yp.tile([P, KVH, YL], BF)
            YV = yp.tile([P, KVH, HD], BF)

            def rstd_from_mv(mv):
                r = statp.tile([P, 1], F32, tag="rstd")
                nc.scalar.activation(out=r, in_=mv[:, 1:2], func=AF.Sqrt,
                                     bias=eps_t, scale=1.0)
                nc.vector.reciprocal(out=r, in_=r)
                return r

            def transpose_to(zn, nheads, sb_dst, tok0):
                """PE-transpose zn's heads ([P tok, nheads*HD] f32) into
                head-major bf16 SBUF dst[:, hg4, tok0:tok0+P]."""
                for hg in range(nheads // 4):
                    tp = psA.tile([P, 4, P], F32, tag="tr", bufs=2)
                    for j in range(4):
                        hh = hg * 4 + j
                        nc.tensor.transpose(
                            tp[:, j, :], zn[:, hh * HD:(hh + 1) * HD], ident)
                    nc.vector.tensor_copy(
                        out=sb_dst[:, hg * 4:(hg + 1) * 4, tok0:tok0 + P],
                        in_=tp)

            def ln_apply_sb(dst, nchunks, w_t, b_t):
                """In-place LN on dst [P, nchunks*NQ] f32 (already in SBUF).
                Spread across engines: stats DVE, normalize ACT, bias GPSIMD."""
                mv = _ln_stats(nc, statp,
                               [dst[:, n * NQ:(n + 1) * NQ]
                                for n in range(nchunks)])
                rstd = rstd_from_mv(mv)
                negmr = statp.tile([P, 1], F32, tag="negmr")
                nc.vector.tensor_scalar(
                    out=negmr, in0=mv[:, 0:1], scalar1=rstd, scalar2=-1.0,
                    op0=ALU.mult, op1=ALU.mult)
                n_tot = nchunks * NQ
                nc.scalar.activation(
                    out=dst[:, :n_tot], in_=dst[:, :n_tot], func=AF.Identity,
                    scale=rstd, bias=negmr)
                nc.vector.tensor_mul(out=dst[:, :n_tot], in0=dst[:, :n_tot],
                                     in1=w_t)
                nc.gpsimd.tensor_add(out=dst[:, :n_tot], in0=dst[:, :n_tot],
                                     in1=b_t)

            def load_cs_table(cos_d, sin_d, nchunks):
                ct = ropep.tile([P, TC, HD // 2], F32, tag="costab", bufs=1)
                st = ropep.tile([P, TC, HD // 2], F32, tag="sintab", bufs=1)
                nc.sync.dma_start(
                    out=ct[:, :nchunks, :],
                    in_=cos_d[:, :].rearrange("(t p) f -> p t f", p=P))
                nc.sync.dma_start(
                    out=st[:, :nchunks, :],
                    in_=sin_d[:, :].rearrange("(t p) f -> p t f", p=P))
                return ct, st

            # =========================================================
            # Stage B: Q proj + LN + rope + transpose -> QT_sb -> spill
            # =========================================================
            qtbp = tc.alloc_tile_pool(name="qtb", bufs=1)
            QT_sb = qtbp.tile([P, H, S_LOC], BF)
            wB = tc.alloc_tile_pool(name="wB", bufs=1)
            wq_sb = wB.tile([P, DC, D], BF, tag="w")
            xq_ap = xTq[:, :].rearrange("(dc p) s -> p dc s", p=P)
            xt_first = xs.tile([P, DC, P], BF, tag="xt", name="xt_first")
            nc.sync.dma_start(out=xt_first, in_=xq_ap[:, :, 0:P])
            wq_ap = wq_d[:, :].rearrange("(dc p) n -> p dc n", p=P)
            for dc in range(DC):
                nc.sync.dma_start(out=wq_sb[:, dc, :], in_=wq_ap[:, dc, :])
            cosq_t, sinq_t = load_cs_table(cosq_d, sinq_d, TCL)
            for tcl in range(TCL):
                tok0 = tcl * P
                if tcl == 0:
                    xt_t = xt_first
                else:
                    xt_t = xs.tile([P, DC, P], BF, tag="xt")
                    nc.sync.dma_start(out=xt_t, in_=xq_ap[:, :, tok0:tok0 + P])
                q_ps = [psA.tile([P, NQ], F32, tag=f"acc{n}", name=f"qps{n}",
                                 bufs=1) for n in range(4)]
                for dc in range(DC):
                    for n in range(4):
                        nc.tensor.matmul(
                            q_ps[n][:], lhsT=xt_t[:, dc, :],
                            rhs=wq_sb[:, dc, n * NQ:(n + 1) * NQ],
                            start=(dc == 0), stop=(dc == DC - 1))
                qn = work.tile([P, D], F32, tag="work")
                for n in range(4):
                    nc.scalar.copy(out=qn[:, n * NQ:(n + 1) * NQ],
                                   in_=q_ps[n][:])
                ln_apply_sb(qn, 4, qw_t, qb_t)
                _rope_inplace(nc, ropep, qn, H, cosq_t[:, tcl, :],
                              sinq_t[:, tcl, :])
                transpose_to(qn, H, QT_sb, tok0)
            for h in range(H):
                nc.sync.dma_start(out=QT_dram[:, h, :], in_=QT_sb[:, h, :])
            wB.release()
            qtbp.release()

            # =========================================================
            # Stage A-K: K projection (full seq) + LN + rope -> spill
            # =========================================================
            x_ap = xT[:, :].rearrange("(dc p) s -> p dc s", p=P)
            ktbp = tc.alloc_tile_pool(name="ktb", bufs=1)
            KT_sb = ktbp.tile([P, KVH, S], BF)
            wAk = tc.alloc_tile_pool(name="wAk", bufs=1)
            wk_sb = wAk.tile([P, DC, KVD], BF, tag="w")
            xt_firstk = xs.tile([P, DC, P], BF, tag="xt", name="xt_firstk")
            nc.sync.dma_start(out=xt_firstk, in_=x_ap[:, :, 0:P])
            wk_ap = wkv_d[:, :KVD].rearrange("(dc p) n -> p dc n", p=P)
            for dc in range(DC):
                nc.sync.dma_start(out=wk_sb[:, dc, :], in_=wk_ap[:, dc, :])
            cosk_t, sink_t = load_cs_table(cosk_d, sink_d, TC)
            for tci in range(TC):
                tok0 = tci * P
                if tci == 0:
                    xt_t = xt_firstk
                else:
                    xt_t = xs.tile([P, DC, P], BF, tag="xt")
                    nc.sync.dma_start(out=xt_t, in_=x_ap[:, :, tok0:tok0 + P])
                k_ps = [psA.tile([P, NQ], F32, tag=f"acc{n}", name=f"kps{n}",
                                 bufs=1) for n in range(2)]
                for dc in range(DC):
                    for n in range(2):
                        nc.tensor.matmul(
                            k_ps[n][:], lhsT=xt_t[:, dc, :],
                            rhs=wk_sb[:, dc, n * NQ:(n + 1) * NQ],
                            start=(dc == 0), stop=(dc == DC - 1))
                kn = work.tile([P, KVD], F32, tag="work")
                for n in range(2):
                    nc.scalar.copy(out=kn[:, n * NQ:(n + 1) * NQ],
                                   in_=k_ps[n][:])
                ln_apply_sb(kn, 2, kw_t, kb_t)
                _rope_inplace(nc, ropep, kn, KVH, cosk_t[:, tci, :],
                              sink_t[:, tci, :])
                transpose_to(kn, KVH, KT_sb, tok0)
            for kv in range(KVH):
                nc.sync.dma_start(out=KT_dram[:, kv, :], in_=KT_sb[:, kv, :])
            wAk.release()
            ktbp.release()

            # =========================================================
            # Stage A-V: V projection (full seq) -> spill
            # =========================================================
            vbp = tc.alloc_tile_pool(name="vb", bufs=1)
            V_sb = vbp.tile([P, TC, KVD], BF)
            wAv = tc.alloc_tile_pool(name="wAv", bufs=1)
            wv_sb = wAv.tile([P, DC, KVD], BF, tag="w")
            xt_firstv = xs.tile([P, DC, P], BF, tag="xt", name="xt_firstv")
            nc.sync.dma_start(out=xt_firstv, in_=x_ap[:, :, 0:P])
            wv_ap = wkv_d[:, KVD:].rearrange("(dc p) n -> p dc n", p=P)
            for dc in range(DC):
                nc.sync.dma_start(out=wv_sb[:, dc, :], in_=wv_ap[:, dc, :])
            for tci in range(TC):
                tok0 = tci * P
                if tci == 0:
                    xt_t = xt_firstv
                else:
                    xt_t = xs.tile([P, DC, P], BF, tag="xt")
                    nc.sync.dma_start(out=xt_t, in_=x_ap[:, :, tok0:tok0 + P])
                v_ps = [psA.tile([P, NQ], F32, tag=f"acc{n}", name=f"vps{n}",
                                 bufs=1) for n in range(2)]
                for dc in range(DC):
                    for n in range(2):
                        nc.tensor.matmul(
                            v_ps[n][:], lhsT=xt_t[:, dc, :],
                            rhs=wv_sb[:, dc, n * NQ:(n + 1) * NQ],
                            start=(dc == 0), stop=(dc == DC - 1))
                for n in range(2):
                    nc.scalar.copy(
                        out=V_sb[:, tci, n * NQ:(n + 1) * NQ], in_=v_ps[n][:])
            for kv in range(KVH):
                nc.sync.dma_start(
                    out=V_dram[:, :, kv * HD:(kv + 1) * HD],
                    in_=V_sb[:, :, kv * HD:(kv + 1) * HD])
            wAv.release()
            vbp.release()

            # =========================================================
            # Stage C: y projections -> YKT (LN, no rope), YV (SBUF)
            # =========================================================
            wC = tc.alloc_tile_pool(name="wC", bufs=1)
            wkvy_sb = wC.tile([P, YDC, 2 * KVD], BF, tag="w")
            nc.sync.dma_start(
                out=wkvy_sb, in_=wkvy_d[:, :].rearrange("(dc p) n -> p dc n", p=P))
            yt_t = xs.tile([P, YDC, YL], BF, tag="yt", bufs=1)
            nc.sync.dma_start(
                out=yt_t, in_=yT[:, :].rearrange("(dc p) s -> p dc s", p=P))
            yk_ps = [psA.tile([P, NQ], F32, tag=f"acc{n}", name=f"ykps{n}",
                              bufs=1) for n in range(2)]
            yv_ps = [psA.tile([P, NQ], F32, tag=f"acc{n+2}", name=f"yvps{n}",
                              bufs=1) for n in range(2)]
            for dc in range(YDC):
                for n in range(2):
                    nc.tensor.matmul(
                        yk_ps[n][:], lhsT=yt_t[:, dc, :],
                        rhs=wkvy_sb[:, dc, n * NQ:(n + 1) * NQ],
                        start=(dc == 0), stop=(dc == YDC - 1))
                for n in range(2):
                    nc.tensor.matmul(
                        yv_ps[n][:], lhsT=yt_t[:, dc, :],
                        rhs=wkvy_sb[:, dc, KVD + n * NQ:KVD + (n + 1) * NQ],
                        start=(dc == 0), stop=(dc == YDC - 1))
            for n in range(2):
                nc.scalar.copy(
                    out=YV[:, 4 * n:4 * (n + 1), :], in_=yv_ps[n][:])
            ykn = work.tile([P, KVD], F32, tag="work")
            for n in range(2):
                nc.scalar.copy(out=ykn[:, n * NQ:(n + 1) * NQ],
                               in_=yk_ps[n][:])
            ln_apply_sb(ykn, 2, kyw_t, kyb_t)
            for hg in range(2):
                tp = psA.tile([P, 4, P], F32, tag="tr", bufs=2)
                for j in range(4):
                    kv = hg * 4 + j
                    nc.tensor.transpose(
                        tp[:, j, :], ykn[:, kv * HD:(kv + 1) * HD], ident)
                nc.vector.tensor_copy(
                    out=YKT[:, hg * 4:(hg + 1) * 4, :], in_=tp)
            wC.release()
            statp.release()
            ropep.release()
            work.release()
            xs.release()
            lnp.release()
            psA.release()

            # =========================================================
            # Stage D: attention per (head, q-chunk)
            # =========================================================
            mgp = tc.alloc_tile_pool(name="merged", bufs=1)
            ktrp = tc.alloc_tile_pool(name="ktr", bufs=1)
            vrp = tc.alloc_tile_pool(name="vr", bufs=1)
            qtrp = tc.alloc_tile_pool(name="qtr", bufs=1)
            ep = tc.alloc_tile_pool(name="escores", bufs=2)
            eyp = tc.alloc_tile_pool(name="eyscores", bufs=2)
            rcp = tc.alloc_tile_pool(name="recips", bufs=2)
            psD = tc.alloc_tile_pool(name="psD", bufs=2, space="PSUM")
            merged = mgp.tile([P, H, S_LOC], BF)      # merged^T feature-major
            KTr = ktrp.tile([P, KVH, S], BF)
            Vr = vrp.tile([P, TC, KVD], BF)
            QTr = qtrp.tile([P, H, S_LOC], BF)
            for kv in range(KVH):
                nc.sync.dma_start(out=KTr[:, kv, :], in_=KT_dram[:, kv, :])
                nc.sync.dma_start(
                    out=Vr[:, :, kv * HD:(kv + 1) * HD],
                    in_=V_dram[:, :, kv * HD:(kv + 1) * HD])
            for h in range(H):
                nc.sync.dma_start(out=QTr[:, h, :], in_=QT_dram[:, h, :])
            for h in range(H):
                kv = h // NREP
                for qc in range(QCN):
                    q0 = qc * NQ
                    qt_t = QTr[:, h, q0:q0 + NQ]
                    # cross-attention first: short chain, overlaps the
                    # self-attention pipeline instead of serializing its tail
                    sy_ps = psD.tile([P, NQ], F32, tag="sy", bufs=1, name="sy_ps")
                    nc.tensor.matmul(
                        sy_ps[:], lhsT=YKT[:, kv, :], rhs=qt_t,
                        start=True, stop=True, skip_group_check=True)
                    Ey_t = eyp.tile([P, NQ], BF, tag="Ey")
                    nc.scalar.activation(
                        out=Ey_t, in_=sy_ps[:], func=AF.Exp, scale=SCALE,
                        bias=ymb_t)
                    dy_ps = psD.tile([P, NQ], F32, tag="cross", bufs=1, name="dy_ps")
                    nc.tensor.matmul(
                        dy_ps[:], lhsT=ones_t, rhs=Ey_t,
                        start=True, stop=True, skip_group_check=True)
                    oy_ps = psD.tile([P, NQ], F32, tag="cross", bufs=1, name="oy_ps")
                    nc.tensor.matmul(
                        oy_ps[:], lhsT=YV[:, kv, :], rhs=Ey_t,
                        start=True, stop=True, skip_group_check=True)
                    rec_y = rcp.tile([P, NQ], F32, tag="recy")
                    nc.vector.reciprocal(out=rec_y, in_=dy_ps[:])
                    t1 = rcp.tile([P, NQ], F32, tag="t1")
                    nc.vector.scalar_tensor_tensor(
                        out=t1, in0=oy_ps[:], scalar=gates_t[:, h:h + 1],
                        in1=rec_y, op0=ALU.mult, op1=ALU.mult)
                    o_ps = psD.tile([P, NQ], F32, tag="o", bufs=1)
                    d_ps = psD.tile([P, NQ], F32, tag="d", bufs=1)
                    E_t = ep.tile([P, TC, NQ], BF, tag="E", bufs=3)
                    for kp in range(TC // 2):
                        s_ps = psD.tile([P, 2, NQ], F32, tag="s", bufs=2)
                        for j in range(2):
                            kc = kp * 2 + j
                            nc.tensor.matmul(
                                s_ps[:, j, :],
                                lhsT=KTr[:, kv, kc * P:(kc + 1) * P],
                                rhs=qt_t, start=True, stop=True,
                                skip_group_check=True)
                        nc.scalar.activation(
                            out=E_t[:, kp * 2:kp * 2 + 2, :], in_=s_ps[:],
                            func=AF.Exp, scale=SCALE)
                        # pairwise E sum on DVE halves the denominator matmuls
                        esum = eyp.tile([P, NQ], BF, tag="es", bufs=4,
                                        name="esum")
                        nc.vector.tensor_add(
                            out=esum, in0=E_t[:, kp * 2, :],
                            in1=E_t[:, kp * 2 + 1, :])
                        nc.tensor.matmul(
                            d_ps[:], lhsT=ones_t, rhs=esum,
                            start=(kp == 0), stop=(kp == TC // 2 - 1),
                            skip_group_check=True)
                        for j in range(2):
                            kc = kp * 2 + j
                            nc.tensor.matmul(
                                o_ps[:], lhsT=Vr[:, kc, kv * HD:(kv + 1) * HD],
                                rhs=E_t[:, kc, :],
                                start=(kc == 0), stop=(kc == TC - 1),
                                skip_group_check=True)
                    # merge: O/denom + tanh(gate)*Oy/denom_y (t1 ready above)
                    rec = rcp.tile([P, NQ], F32, tag="rec")
                    nc.vector.reciprocal(out=rec, in_=d_ps[:])
                    t0 = rcp.tile([P, NQ], F32, tag="t0")
                    nc.vector.tensor_mul(out=t0, in0=o_ps[:], in1=rec)
                    nc.vector.tensor_add(
                        out=merged[:, h, q0:q0 + NQ], in0=t0, in1=t1)
            rcp.release()
            eyp.release()
            ep.release()
            qtrp.release()
            vrp.release()
            ktrp.release()
            psD.release()

            # =========================================================
            # Stage E: output projection out^T = wo^T @ merged^T
            # =========================================================
            wE = tc.alloc_tile_pool(name="wE", bufs=1)
            outp = tc.alloc_tile_pool(name="outtiles", bufs=3)
            psE = tc.alloc_tile_pool(name="psE", bufs=2, space="PSUM")
            wo_sb = wE.tile([P, DC, D], BF, tag="w")
            wo_ap = wo_d[:, :].rearrange("(hc p) n -> p hc n", p=P)
            for hc in range(DC):
                nc.sync.dma_start(out=wo_sb[:, hc, :], in_=wo_ap[:, hc, :])
            for qc in range(QCN):
                q0 = qc * NQ
                for oc in range(DC):
                    out_ps = psE.tile([P, NQ], F32, tag="oout")
                    for hc in range(DC):
                        nc.tensor.matmul(
                            out_ps[:],
                            lhsT=wo_sb[:, hc, oc * P:(oc + 1) * P],
                            rhs=merged[:, hc, q0:q0 + NQ],
                            start=(hc == 0), stop=(hc == DC - 1))
                    out_t = outp.tile([P, NQ], F32, tag="outt")
                    nc.vector.tensor_copy(out=out_t, in_=out_ps[:])
                    nc.sync.dma_start(
                        out=outT[oc * P:(oc + 1) * P, q0:q0 + NQ],
                        in_=out_t)
            psE.release()
            outp.release()
            wE.release()
            mgp.release()

    _split_dma_waits(nc)
    return nc


def _prep_shared(x, y, freqs_cos, freqs_sin, y_mask, wq, wk, wv, wk_y, wv_y,
                 wo, q_w, q_b, k_w, k_b, ky_w, ky_b, gate):
    f32 = np.float32
    shared = {
        "wq": np.ascontiguousarray(np.asarray(wq, f32).astype(BF16)),
        "wkv": np.ascontiguousarray(
            np.concatenate([np.asarray(wk, f32), np.asarray(wv, f32)],
                           axis=1).astype(BF16)),
        "wkvy": np.ascontiguousarray(
            np.concatenate([np.asarray(wk_y, f32), np.asarray(wv_y, f32)],
                           axis=1).astype(BF16)),
        "wo": np.ascontiguousarray(np.asarray(wo, f32).astype(BF16)),
        "qw": np.ascontiguousarray(np.asarray(q_w, f32)),
        "qb": np.ascontiguousarray(np.asarray(q_b, f32)),
        "kw": np.ascontiguousarray(np.asarray(k_w, f32)),
        "kb": np.ascontiguousarray(np.asarray(k_b, f32)),
        "kyw": np.ascontiguousarray(np.asarray(ky_w, f32)),
        "kyb": np.ascontiguousarray(np.asarray(ky_b, f32)),
        "cosk": np.ascontiguousarray(np.asarray(freqs_cos, f32)),
        "sink": np.ascontiguousarray(np.asarray(freqs_sin, f32)),
        "gates": np.ascontiguousarray(np.tanh(np.asarray(gate, f32))),
    }
    per_core = []
    for c in range(8):
        b, hf = c // 2, c % 2
        sl = slice(hf * S_LOC, (hf + 1) * S_LOC)
        xTb = np.asarray(x[b], f32).T.astype(BF16)
        m = dict(shared)
        m["xT"] = np.ascontiguousarray(xTb)
        m["xTq"] = np.ascontiguousarray(xTb[:, sl])
        m["yT"] = np.ascontiguousarray(np.asarray(y[b], f32).T.astype(BF16))
        m["cosq"] = np.ascontiguousarray(np.asarray(freqs_cos, f32)[sl])
        m["sinq"] = np.ascontiguousarray(np.asarray(freqs_sin, f32)[sl])
        m["ymb"] = np.where(np.asarray(y_mask[b]), 0.0, -1e9).astype(f32)
        per_core.append(m)
    return per_core


def kernel(**inputs):
    if "nc" not in _CACHED:
        _CACHED["nc"] = build_program()
    nc = _CACHED["nc"]
    in_maps = _prep_shared(
        inputs["x"], inputs["y"], inputs["freqs_cos"], inputs["freqs_sin"],
        inputs["y_mask"], inputs["wq"], inputs["wk"], inputs["wv"],
        inputs["wk_y"], inputs["wv_y"], inputs["wo"], inputs["q_w"],
        inputs["q_b"], inputs["k_w"], inputs["k_b"], inputs["ky_w"],
        inputs["ky_b"], inputs["gate"])
    res = run_bass_kernel_spmd(nc, in_maps, core_ids=list(range(8)))
    global LAST_EXEC_NS
    LAST_EXEC_NS = res.exec_time_ns
    out = np.zeros((B, S, D), np.float32)
    for c in range(8):
        b, hf = c // 2, c % 2
        out[b, hf * S_LOC:(hf + 1) * S_LOC, :] = res.results[c]["outT"].T
    return out


if __name__ == "__main__":
    nc = build_program()
    print("program built OK")



# revision 16
# speedup vs baseline: 1.1364x; 1.1364x over previous
"""Trainium2 Bass kernel for nn_Attention_35734127903400 (v2.2).

Dense transformer attention block:
  xq = LN(x@wq); xk = LN(x@wk); xv = x@wv          (LN over full flattened head dim)
  rope(q, k); GQA self-attention (16 q heads, 8 kv heads, S=2048, full/non-causal)
  gated cross-attention with y (128 tokens); out = (self + tanh(gate)*cross) @ wo

Sharding (8 cores, no collectives): token-sharded. Core c handles batch
b=c//2, sequence half hf=c%2 (1024 q tokens). Each core computes K/V for
its batch's FULL 2048-token sequence (replicated within the pair), Q only
for its local 1024 tokens. LN is over the feature dim so it is core-local.

Everything stays resident in SBUF — no DRAM spill/reload. Stage order:
y proj (startup filler while wq streams) -> Q proj -> K+V proj (fused,
one x pass) -> attention -> output proj (wo streamed per output chunk).

Scheduling specifics, tuned against the TimelineSim cost model:
- proj chunks: the final contraction round runs the single-buffered
  accumulators first and the PSUM->SBUF copies are emitted in the same
  order, so the next chunk's matmuls never wait on a copy.
- LN affine (Pool) + rope (DVE) run per head-half; head transposes (PE,
  bf16 = 1 cycle/row) trail by 1 chunk for half 0 and 2 chunks for
  half 1, hiding the whole post-processing chain.
- attention: softmax denominator via DVE/Pool pairwise-add tree feeding
  5 ones-matmuls spread through the AV stream; cross-attention matmuls
  (dy/oy) slotted where their PSUM bank is free; exp is the pacing
  engine (ACT) and everything else rides in its shadow.
- weight DMAs are per-dc so matmuls start after the first slice; wkv
  prefetches into an untouched right-side SBUF region during Q so the
  KV stage starts without a DMA bubble.
"""

import numpy as np
import ml_dtypes

import concourse.bass as bass
import concourse.mybir as mybir
import concourse.tile as tile
from concourse.bass_utils import run_bass_kernel_spmd
from concourse.masks import make_identity

BF16 = ml_dtypes.bfloat16
F32 = mybir.dt.float32
BF = mybir.dt.bfloat16

P = 128
B, S, D = 4, 2048, 2048
H, KVH = 16, 8
HD = 128
NREP = 2
YL, YD = 128, 1024
EPS = 1e-5
S_LOC = S // 2
DC = D // P          # 16 contraction chunks for D
YDC = YD // P        # 8
TC = S // P          # 16 token chunks (full seq)
TCL = S_LOC // P     # 8 local token chunks
NQ = 512
QCN = S_LOC // NQ    # 2
KVD = KVH * HD       # 1024
SCALE = 1.0 / float(np.sqrt(np.float32(HD)))
AF = mybir.ActivationFunctionType
ALU = mybir.AluOpType

_CACHED = {}
LAST_EXEC_NS = None


def _split_dma_waits(nc, max_waits=1):
    """Hoist excess sync-waits of any instruction onto preceding same-engine
    single-wait NoOps (this build's per-instruction structs have few embedded
    wait slots)."""
    n_split = 0
    for f in nc.m.functions:
        for blk in f.blocks:
            insts = list(blk.instructions)
            out = []
            changed = False
            for ins in insts:
                si = ins.sync_info
                if (si is not None and si.on_wait
                        and len(si.on_wait) > max_waits):
                    waits = list(si.on_wait)
                    for wi, w in enumerate(waits[:-max_waits]):
                        out.append(mybir.InstNoOp(
                            name=f"{ins.name}-wsplit{wi}", engine=ins.engine,
                            sync_info=mybir.SyncInfo(on_wait=[w],
                                                     on_update=[])))
                    ins.sync_info = mybir.SyncInfo(
                        on_wait=waits[-max_waits:],
                        on_update=list(si.on_update))
                    changed = True
                    n_split += 1
                out.append(ins)
            if changed:
                blk.instructions = out
    return n_split


def build_program():
    nc = bass.Bass()

    # ---- I/O (all pre-chunked host-side for >=2KB contiguous runs) ----
    xq_d = nc.declare_dram_parameter("xq", [TCL, P, DC, P], BF, isOutput=False)
    x_d = nc.declare_dram_parameter("x", [TC, P, DC, P], BF, isOutput=False)
    y_d = nc.declare_dram_parameter("y", [P, YDC, YL], BF, isOutput=False)
    wq_d = nc.declare_dram_parameter("wq", [DC, P, D], BF, isOutput=False)
    wkv_d = nc.declare_dram_parameter("wkv", [DC, P, 2 * KVD], BF,
                                      isOutput=False)
    wkvy_d = nc.declare_dram_parameter("wkvy", [YDC, P, 2 * KVD], BF,
                                       isOutput=False)
    wo_d = nc.declare_dram_parameter("wo", [DC, P, DC, P], BF, isOutput=False)
    qw_d = nc.declare_dram_parameter("qw", [D], F32, isOutput=False)
    qb_d = nc.declare_dram_parameter("qb", [D], F32, isOutput=False)
    kw_d = nc.declare_dram_parameter("kw", [KVD], F32, isOutput=False)
    kb_d = nc.declare_dram_parameter("kb", [KVD], F32, isOutput=False)
    kyw_d = nc.declare_dram_parameter("kyw", [KVD], F32, isOutput=False)
    kyb_d = nc.declare_dram_parameter("kyb", [KVD], F32, isOutput=False)
    cosq_d = nc.declare_dram_parameter("cosq", [P, TCL, HD // 2], F32,
                                       isOutput=False)
    sinq_d = nc.declare_dram_parameter("sinq", [P, TCL, HD // 2], F32,
                                       isOutput=False)
    cosk_d = nc.declare_dram_parameter("cosk", [P, TC, HD // 2], F32,
                                       isOutput=False)
    sink_d = nc.declare_dram_parameter("sink", [P, TC, HD // 2], F32,
                                       isOutput=False)
    gates_d = nc.declare_dram_parameter("gates", [H], F32, isOutput=False)
    ymb_d = nc.declare_dram_parameter("ymb", [YL], F32, isOutput=False)
    outT = nc.declare_dram_parameter("outT", [D, S_LOC], F32, isOutput=True)

    with tile.TileContext(nc) as tc:
        from contextlib import ExitStack
        with ExitStack() as ctx:
            cpool = ctx.enter_context(tc.tile_pool(name="consts", bufs=1))
            ident = cpool.tile([P, P], BF)
            make_identity(nc, ident)
            ones_t = cpool.tile([P, P], BF)
            nc.vector.memset(ones_t, 1.0)
            eps_t = cpool.tile([P, 1], F32)
            nc.vector.memset(eps_t, EPS)
            gates_t = cpool.tile([P, H], F32)
            nc.gpsimd.dma_start(
                out=gates_t,
                in_=bass.AP(tensor=gates_d, offset=0, ap=[[0, P], [1, H]]))
            ymb_t = cpool.tile([P, 1], F32)
            nc.gpsimd.dma_start(
                out=ymb_t,
                in_=bass.AP(tensor=ymb_d, offset=0, ap=[[1, P], [0, 1]]))

            def bcast_vec(pool, dram_h, n):
                t = pool.tile([P, n], F32, tag=f"ln_{dram_h.name}", bufs=1)
                nc.gpsimd.dma_start(
                    out=t,
                    in_=bass.AP(tensor=dram_h, offset=0, ap=[[0, P], [1, n]]))
                return t

            def ln_chain(zn, nln, pool, w_t, b_t, pfx):
                """stats (DVE) -> rstd/negmr -> normalize (ACT). Affine is
                applied by the caller (per-half on Pool)."""
                stats = pool.tile([P, nln, 6], F32, tag=f"{pfx}bnstats")
                for i in range(nln):
                    nc.vector.bn_stats(out=stats[:, i, :],
                                       in_=zn[:, i * NQ:(i + 1) * NQ])
                mv = pool.tile([P, 2], F32, tag=f"{pfx}bnaggr")
                nc.vector.bn_aggr(out=mv, in_=stats)
                rstd = pool.tile([P, 1], F32, tag=f"{pfx}rstd")
                nc.scalar.activation(out=rstd, in_=mv[:, 1:2],
                                     func=AF.Sqrt, bias=eps_t, scale=1.0)
                nc.vector.reciprocal(out=rstd, in_=rstd)
                negmr = pool.tile([P, 1], F32, tag=f"{pfx}negmr")
                nc.vector.tensor_scalar(
                    out=negmr, in0=mv[:, 0:1], scalar1=rstd, scalar2=-1.0,
                    op0=ALU.mult, op1=ALU.mult)
                nc.scalar.activation(out=zn, in_=zn, func=AF.Identity,
                                     scale=rstd, bias=negmr)

            # =========================================================
            # Stage Y: y projections -> YKT (LN, no rope), YV.
            # Runs first: its matmuls fill the PE while wq streams in.
            # =========================================================
            yp = ctx.enter_context(tc.tile_pool(name="ypool", bufs=1))
            YKT = yp.tile([P, KVH, YL], BF)
            YV = yp.tile([P, KVH, HD], BF)
            lny = tc.alloc_tile_pool(name="lny", bufs=1)
            wY = tc.alloc_tile_pool(name="wY", bufs=1)
            yt = wY.tile([P, YDC, YL], BF, tag="yt")
            nc.sync.dma_start(out=yt, in_=y_d[:, :, :])
            wy_sb = []
            for g in range(YDC):
                wt = wY.tile([P, 2 * KVD], BF, tag=f"wy{g}", name=f"wy{g}")
                nc.sync.dma_start(out=wt, in_=wkvy_d[g])
                wy_sb.append(wt)
            kyw_t = bcast_vec(wY, kyw_d, KVD)
            kyb_t = bcast_vec(wY, kyb_d, KVD)
            psY = tc.alloc_tile_pool(name="psY", bufs=1, space="PSUM")
            ya = [psY.tile([P, NQ], F32, tag=f"ya{n}", bufs=1, name=f"ya{n}")
                  for n in range(4)]
            for dc in range(YDC):
                for n in range(4):
                    nc.tensor.matmul(
                        ya[n][:], lhsT=yt[:, dc, :],
                        rhs=wy_sb[dc][:, n * NQ:(n + 1) * NQ],
                        start=(dc == 0), stop=(dc == YDC - 1))
            ykn = wY.tile([P, KVD], F32, tag="ykn")
            for n in range(2):
                nc.scalar.copy(out=ykn[:, n * NQ:(n + 1) * NQ], in_=ya[n][:])
                nc.scalar.copy(out=YV[:, 4 * n:4 * (n + 1), :],
                               in_=ya[2 + n][:])
            ln_chain(ykn, 2, wY, kyw_t, kyb_t, "y")
            nc.gpsimd.tensor_mul(out=ykn, in0=ykn, in1=kyw_t)
            nc.gpsimd.tensor_add(out=ykn, in0=ykn, in1=kyb_t)
            ykbf = wY.tile([P, KVH, HD], BF, tag="ykbf")
            nc.vector.tensor_copy(out=ykbf, in_=ykn)
            for hg in range(2):
                tp = psY.tile([P, 4, P], BF, tag="ytr", bufs=2)
                for j in range(4):
                    nc.tensor.transpose(
                        tp[:, j, :], ykbf[:, hg * 4 + j, :], ident)
                nc.scalar.copy(
                    out=YKT[:, hg * 4:(hg + 1) * 4, :], in_=tp)
            psY.release()
            wY.release()
            lny.release()

            qtp = ctx.enter_context(tc.tile_pool(name="qtpool", bufs=1))
            QT = qtp.tile([P, H, S_LOC], BF)

            # =========================================================
            # shared projection-stage machinery
            # =========================================================
            # final-contraction-round matmul order / copy order: the
            # single-buffered accumulators (2, 3) finish and copy first
            ACC_ORDER = [2, 3, 0, 1]

            def proj_stage(nchunks, x_dram, w_tiles, out_heads,
                           w_t, b_t, cos_dram, sin_dram, dst_T, dst_V, stage,
                           xs, preloaded, hooks):
                """One pass over `nchunks` token chunks with 4 accumulators.

                The first `out_heads*HD` features get LN+rope+transpose into
                dst_T; for the KV stage accumulators [2,3] are V, copied raw
                into dst_V[:, chunk, :]. `xs`: caller-owned x-tile pool
                (chunks in `preloaded` were DMA'd by the caller before the
                weight DMAs). `hooks[t]` runs after chunk t's x DMA — used to
                interleave next-stage prefetch DMAs into the SP queue.
                """
                nacc = 4
                csp = tc.alloc_tile_pool(name=f"cs{stage}", bufs=1)
                cs_tiles = {}
                nhalves = (nchunks + 7) // 8

                def load_cs_half(hh):
                    ct = csp.tile([P, 8, HD // 2], F32, tag="ctab",
                                  bufs=nhalves, name=f"ctab{stage}_{hh}")
                    st = csp.tile([P, 8, HD // 2], F32, tag="stab",
                                  bufs=nhalves, name=f"stab{stage}_{hh}")
                    nc.sync.dma_start(out=ct,
                                      in_=cos_dram[:, 8 * hh:8 * hh + 8, :])
                    nc.sync.dma_start(out=st,
                                      in_=sin_dram[:, 8 * hh:8 * hh + 8, :])
                    cs_tiles[hh] = (ct, st)

                load_cs_half(0)
                wk_ = tc.alloc_tile_pool(name=f"work{stage}", bufs=2)
                bfp = tc.alloc_tile_pool(name=f"bf{stage}", bufs=3)
                stp = tc.alloc_tile_pool(name=f"st{stage}", bufs=2)
                rtp = tc.alloc_tile_pool(name=f"rt{stage}", bufs=1)
                psP = tc.alloc_tile_pool(name=f"ps{stage}", bufs=1,
                                         space="PSUM")
                nfeat = out_heads * HD
                nln = nfeat // NQ        # accumulators covered by LN
                oh2 = out_heads // 2     # heads per half
                pending = []             # [(zbf, tok0)]

                def emit_transposes(zbf, tok0):
                    for hg in range(out_heads // 4):
                        h0 = hg * 4
                        tp = psP.tile([P, 4, P], BF, tag="tr", bufs=2)
                        for j in range(4):
                            nc.tensor.transpose(
                                tp[:, j, :], zbf[:, h0 + j, :], ident)
                        nc.scalar.copy(
                            out=dst_T[:, h0:h0 + 4, tok0:tok0 + P],
                            in_=tp)

                def drain(n):
                    for _ in range(n):
                        if pending:
                            emit_transposes(*pending.pop(0))

                for t in range(nchunks):
                    if t in preloaded:
                        xt = preloaded[t]
                    else:
                        xt = xs.tile([P, DC, P], BF, tag="xt",
                                     name=f"xt{stage}_{t}")
                        nc.sync.dma_start(out=xt, in_=x_dram[t])
                    if t in hooks:
                        hooks[t]()
                    if (t % 8 == 6 and t + 2 < nchunks
                            and (t + 2) // 8 not in cs_tiles):
                        load_cs_half((t + 2) // 8)
                    accs = [psP.tile([P, NQ], F32, tag=f"acc{n}",
                                     bufs=(2 if n < 2 else 1),
                                     name=f"acc{n}_{t}")
                            for n in range(nacc)]
                    for dc in range(DC - 1):
                        for n in range(nacc):
                            nc.tensor.matmul(
                                accs[n][:], lhsT=xt[:, dc, :],
                                rhs=w_tiles[dc][:, n * NQ:(n + 1) * NQ],
                                start=(dc == 0), stop=False)
                    for n in ACC_ORDER:
                        nc.tensor.matmul(
                            accs[n][:], lhsT=xt[:, DC - 1, :],
                            rhs=w_tiles[DC - 1][:, n * NQ:(n + 1) * NQ],
                            start=False, stop=True)
                    # PSUM -> SBUF copies, staggered order matching the
                    # final round so the next chunk never waits
                    zn = wk_.tile([P, nfeat], F32, tag="work")
                    for n in ACC_ORDER:
                        if n < nln:
                            nc.scalar.copy(out=zn[:, n * NQ:(n + 1) * NQ],
                                           in_=accs[n][:])
                        elif dst_V is not None:
                            nc.scalar.copy(
                                out=dst_V[:, t, (n - nln) * NQ:
                                          (n - nln + 1) * NQ],
                                in_=accs[n][:])
                    # transposes of chunk t-2 (rope long finished)
                    if t >= 2:
                        drain(1)
                    ln_chain(zn, nln, stp, w_t, b_t, stage)
                    # per-half: affine on Pool, rope on DVE -> bf16
                    zbf = bfp.tile([P, out_heads, HD], BF, tag="zbf")
                    zv = zn.rearrange("p (h f two) -> p h f two",
                                      h=out_heads, two=2)
                    zb = zbf.rearrange("p h (f two) -> p h f two", two=2)
                    ct_t, st_t = cs_tiles[t // 8]
                    shp = (P, oh2, HD // 2)
                    cb = ct_t[:, t % 8, :][:, None, :].to_broadcast(shp)
                    sb = st_t[:, t % 8, :][:, None, :].to_broadcast(shp)
                    for half in range(2):
                        f0 = half * (nfeat // 2)
                        f1 = (half + 1) * (nfeat // 2)
                        nc.gpsimd.tensor_mul(out=zn[:, f0:f1],
                                             in0=zn[:, f0:f1],
                                             in1=w_t[:, f0:f1])
                        nc.gpsimd.tensor_add(out=zn[:, f0:f1],
                                             in0=zn[:, f0:f1],
                                             in1=b_t[:, f0:f1])
                        h0, h1 = half * oh2, (half + 1) * oh2
                        re = zv[:, h0:h1, :, 0]
                        im = zv[:, h0:h1, :, 1]
                        rebf = zb[:, h0:h1, :, 0]
                        imbf = zb[:, h0:h1, :, 1]
                        t1 = rtp.tile([P, oh2, HD // 2], F32, tag="r1")
                        t2 = rtp.tile([P, oh2, HD // 2], F32, tag="r2")
                        nc.vector.tensor_mul(out=t1, in0=re, in1=cb)
                        nc.vector.tensor_mul(out=t2, in0=im, in1=sb)
                        nc.vector.tensor_sub(out=rebf, in0=t1, in1=t2)
                        nc.vector.tensor_mul(out=t1, in0=re, in1=sb)
                        nc.vector.tensor_mul(out=t2, in0=im, in1=cb)
                        nc.vector.tensor_add(out=imbf, in0=t1, in1=t2)
                    pending.append((zbf, t * P))
                while pending:
                    drain(1)
                for pool in (psP, rtp, stp, bfp, wk_, csp):
                    pool.release()

            # =========================================================
            # Stage Q: local-half Q projection
            # =========================================================
            lnq = tc.alloc_tile_pool(name="lnq", bufs=1)
            qw_t = bcast_vec(lnq, qw_d, D)
            qb_t = bcast_vec(lnq, qb_d, D)
            xsQ = tc.alloc_tile_pool(name="xsQ", bufs=2)
            xq0 = xsQ.tile([P, DC, P], BF, tag="xt", name="xtQ_0")
            nc.sync.dma_start(out=xq0, in_=xq_d[0])
            xq1 = xsQ.tile([P, DC, P], BF, tag="xt", name="xtQ_1")
            nc.sync.dma_start(out=xq1, in_=xq_d[1])
            wQ = tc.alloc_tile_pool(name="wQ", bufs=1)
            wq_sb = []
            for g in range(DC):
                wt = wQ.tile([P, D], BF, tag=f"wq{g}", name=f"wq{g}")
                nc.sync.dma_start(out=wt, in_=wq_d[g])
                wq_sb.append(wt)

            # prefetch first half of wkv into untouched right-side SBUF
            wKVa = tc.alloc_tile_pool(name="wKVa", bufs=1, side="right")
            wkv_sb = [None] * DC

            def hook_wkva():
                for g in range(8):
                    wt = wKVa.tile([P, 2 * KVD], BF, tag=f"wkv{g}",
                                   name=f"wkv{g}")
                    nc.sync.dma_start(out=wt, in_=wkv_d[g])
                    wkv_sb[g] = wt

            proj_stage(TCL, xq_d, wq_sb, H, qw_t, qb_t,
                       cosq_d, sinq_d, QT, None, "Q",
                       xsQ, {0: xq0, 1: xq1}, {3: hook_wkva})
            wQ.release()
            xsQ.release()
            lnq.release()

            # =========================================================
            # Stage KV: full-seq K (LN+rope) and V projections, one x pass
            # =========================================================
            ktvp = ctx.enter_context(tc.tile_pool(name="ktvpool", bufs=1))
            KT = ktvp.tile([P, KVH, S], BF)
            Vsb = ktvp.tile([P, TC, KVD], BF)
            lnk = tc.alloc_tile_pool(name="lnk", bufs=1)
            kw_t = bcast_vec(lnk, kw_d, KVD)
            kb_t = bcast_vec(lnk, kb_d, KVD)
            # x tiles ahead of the wkv-second-half DMAs in the SP queue
            xsK = tc.alloc_tile_pool(name="xsK", bufs=2)
            xk0 = xsK.tile([P, DC, P], BF, tag="xt", name="xtK_0")
            nc.sync.dma_start(out=xk0, in_=x_d[0])
            xk1 = xsK.tile([P, DC, P], BF, tag="xt", name="xtK_1")
            nc.sync.dma_start(out=xk1, in_=x_d[1])
            # second wkv half into fresh right-side space: its DMAs have no
            # space-dependency on the Q stage and start immediately
            wKVb = tc.alloc_tile_pool(name="wKVb", bufs=1, side="right")
            for g in range(8, DC):
                wt = wKVb.tile([P, 2 * KVD], BF, tag=f"wkv{g}",
                               name=f"wkv{g}")
                nc.sync.dma_start(out=wt, in_=wkv_d[g])
                wkv_sb[g] = wt
            proj_stage(TC, x_d, wkv_sb, KVH, kw_t, kb_t,
                       cosk_d, sink_d, KT, Vsb, "K",
                       xsK, {0: xk0, 1: xk1}, {})
            xsK.release()
            lnk.release()
            wKVb.release()
            wKVa.release()

            # =========================================================
            # Stage attention: per (head, q-chunk)
            # =========================================================
            mgp = ctx.enter_context(tc.tile_pool(name="merged", bufs=1))
            merged = mgp.tile([P, H, S_LOC], BF)
            wop = tc.alloc_tile_pool(name="wop", bufs=3)
            ep = tc.alloc_tile_pool(name="epool", bufs=5)
            esp = tc.alloc_tile_pool(name="espool", bufs=2)
            eyp = tc.alloc_tile_pool(name="eypool", bufs=2)
            rcp = tc.alloc_tile_pool(name="rcpool", bufs=1)
            psA = tc.alloc_tile_pool(name="psA", bufs=1, space="PSUM")
            outp = tc.alloc_tile_pool(name="outp", bufs=3)
            # prefetch first wo slices during attention
            wo_tiles = {}
            for oc in range(2):
                wo_t = wop.tile([P, DC, P], BF, tag="wo", name=f"wo{oc}")
                nc.sync.dma_start(out=wo_t, in_=wo_d[oc])
                wo_tiles[oc] = wo_t

            def emit_outproj_block(oc, qc):
                """16 matmuls into an o-ring PSUM slot + DVE copy + DMA out.
                ACT-free so it absorbs the exp backlog when interleaved."""
                if oc in wo_tiles:
                    wo_t = wo_tiles.pop(oc)
                else:
                    wo_t = wop.tile([P, DC, P], BF, tag="wo",
                                    name=f"wo{oc}_{qc}")
                    nc.sync.dma_start(out=wo_t, in_=wo_d[oc])
                q0 = qc * NQ
                out_ps = psA.tile([P, NQ], F32, tag="o", bufs=2,
                                  name=f"ops{oc}_{qc}")
                for hc in range(DC):
                    nc.tensor.matmul(
                        out_ps[:], lhsT=wo_t[:, hc, :],
                        rhs=merged[:, hc, q0:q0 + NQ],
                        start=(hc == 0), stop=(hc == DC - 1))
                out_t = outp.tile([P, NQ], F32, tag="outt")
                nc.vector.tensor_copy(out=out_t, in_=out_ps[:])
                nc.sync.dma_start(
                    out=outT[oc * P:(oc + 1) * P, q0:q0 + NQ],
                    in_=out_t)

            # cross scores for iteration idx+1 are issued at g5 of idx so
            # ACT's Ey exp never waits on a cold sy matmul
            def emit_sy(idx):
                qc, h = divmod(idx, H)
                kv = h // NREP
                sy = psA.tile([P, NQ], F32, tag="cr", bufs=1,
                              name=f"sy{idx}")
                nc.tensor.matmul(sy[:], lhsT=YKT[:, kv, :],
                                 rhs=QT[:, h, qc * NQ:qc * NQ + NQ],
                                 start=True, stop=True,
                                 skip_group_check=True)
                Ey = eyp.tile([P, NQ], BF, tag="Ey", name=f"Ey{idx}")
                nc.scalar.activation(out=Ey, in_=sy[:], func=AF.Exp,
                                     scale=SCALE, bias=ymb_t)
                return Ey

            NIT = QCN * H
            Ey = emit_sy(0)
            for idx in range(NIT):
                qc, h = divmod(idx, H)
                kv = h // NREP
                q0 = qc * NQ
                qt = QT[:, h, q0:q0 + NQ]
                o_ps = psA.tile([P, NQ], F32, tag="o", bufs=2)
                d_ps = psA.tile([P, NQ], F32, tag="d", bufs=1)
                es8 = esp.tile([P, 8, NQ], BF, tag="es8")
                for g in range(8):
                    s_ps = psA.tile([P, 2, NQ], F32, tag="s", bufs=2)
                    for j in range(2):
                        kc = 2 * g + j
                        nc.tensor.matmul(
                            s_ps[:, j, :],
                            lhsT=KT[:, kv, kc * P:(kc + 1) * P],
                            rhs=qt, start=True, stop=True,
                            skip_group_check=True)
                    E = ep.tile([P, 2, NQ], BF, tag="E")
                    nc.scalar.activation(out=E, in_=s_ps[:],
                                         func=AF.Exp, scale=SCALE)
                    nc.vector.tensor_add(out=es8[:, g, :],
                                         in0=E[:, 0, :], in1=E[:, 1, :])
                    if g in (1, 3, 5):
                        # lvl1 tree add on Pool (in-place into es8[0:3])
                        i = g // 2
                        nc.gpsimd.tensor_add(out=es8[:, i, :],
                                             in0=es8[:, 2 * i, :],
                                             in1=es8[:, 2 * i + 1, :])
                    for j in range(2):
                        kc = 2 * g + j
                        nc.tensor.matmul(
                            o_ps[:],
                            lhsT=Vsb[:, kc, kv * HD:(kv + 1) * HD],
                            rhs=E[:, j, :],
                            start=(kc == 0), stop=(kc == TC - 1),
                            skip_group_check=True)
                    if g == 1:
                        dy = psA.tile([P, NQ], F32, tag="cr", bufs=1,
                                      name=f"dy{idx}")
                        nc.tensor.matmul(dy[:], lhsT=ones_t, rhs=Ey,
                                         start=True, stop=True,
                                         skip_group_check=True)
                        rec_y = rcp.tile([P, NQ], F32, tag="recy")
                        nc.vector.reciprocal(out=rec_y, in_=dy[:])
                    elif g == 3:
                        nc.tensor.matmul(
                            d_ps[:], lhsT=ones_t, rhs=es8[:, 0, :],
                            start=True, stop=False,
                            skip_group_check=True)
                        oy = psA.tile([P, NQ], F32, tag="cr", bufs=1,
                                      name=f"oy{idx}")
                        nc.tensor.matmul(oy[:], lhsT=YV[:, kv, :],
                                         rhs=Ey, start=True, stop=True,
                                         skip_group_check=True)
                        t1 = rcp.tile([P, NQ], F32, tag="t1", bufs=2)
                        nc.vector.scalar_tensor_tensor(
                            out=t1, in0=oy[:],
                            scalar=gates_t[:, h:h + 1],
                            in1=rec_y, op0=ALU.mult, op1=ALU.mult)
                    elif g == 5:
                        nc.tensor.matmul(
                            d_ps[:], lhsT=ones_t, rhs=es8[:, 1, :],
                            start=False, stop=False,
                            skip_group_check=True)
                        if idx + 1 < NIT:
                            next_Ey = emit_sy(idx + 1)
                nc.tensor.matmul(d_ps[:], lhsT=ones_t, rhs=es8[:, 2, :],
                                 start=False, stop=False,
                                 skip_group_check=True)
                nc.tensor.matmul(d_ps[:], lhsT=ones_t, rhs=es8[:, 6, :],
                                 start=False, stop=False,
                                 skip_group_check=True)
                nc.tensor.matmul(d_ps[:], lhsT=ones_t, rhs=es8[:, 7, :],
                                 start=False, stop=True,
                                 skip_group_check=True)
                rec = rcp.tile([P, NQ], F32, tag="rec")
                nc.vector.reciprocal(out=rec, in_=d_ps[:])
                t0 = rcp.tile([P, NQ], F32, tag="t0")
                nc.vector.tensor_mul(out=t0, in0=o_ps[:], in1=rec)
                nc.gpsimd.tensor_add(out=merged[:, h, q0:q0 + NQ],
                                     in0=t0, in1=t1)
                Ey = next_Ey
                # during the second q-chunk pass, interleave the first
                # q-chunk's output projection (ACT-free PE work)
                if qc == 1:
                    emit_outproj_block(h, 0)

            # remaining out-proj: second q-chunk
            for oc in range(DC):
                emit_outproj_block(oc, 1)
            psA.release()
            outp.release()
            rcp.release()
            eyp.release()
            esp.release()
            ep.release()
            wop.release()

    _split_dma_waits(nc)
    return nc


def _prep_inputs(x, y, freqs_cos, freqs_sin, y_mask, wq, wk, wv, wk_y, wv_y,
                 wo, q_w, q_b, k_w, k_b, ky_w, ky_b, gate):
    f32 = np.float32

    def chunk_x(xb):
        # [S, D] -> [tc, p, dc, s]: out[t, p, dc, s] = xb[t*128+s, dc*128+p]
        t = xb.shape[0] // P
        return np.ascontiguousarray(
            xb.reshape(t, P, DC, P).transpose(0, 3, 2, 1).astype(BF16))

    def chunk_cs(tab):
        # [S', 64] -> [p, t, f]
        t = tab.shape[0] // P
        return np.ascontiguousarray(
            np.asarray(tab, f32).reshape(t, P, HD // 2).transpose(1, 0, 2))

    wo_f = np.asarray(wo, f32)
    shared = {
        "wq": np.ascontiguousarray(
            np.asarray(wq, f32).astype(BF16).reshape(DC, P, D)),
        "wkv": np.ascontiguousarray(np.concatenate(
            [np.asarray(wk, f32), np.asarray(wv, f32)],
            axis=1).astype(BF16).reshape(DC, P, 2 * KVD)),
        "wkvy": np.ascontiguousarray(np.concatenate(
            [np.asarray(wk_y, f32), np.asarray(wv_y, f32)],
            axis=1).astype(BF16).reshape(YDC, P, 2 * KVD)),
        "wo": np.ascontiguousarray(
            wo_f.reshape(DC, P, DC, P).transpose(2, 1, 0, 3).astype(BF16)),
        "qw": np.ascontiguousarray(np.asarray(q_w, f32)),
        "qb": np.ascontiguousarray(np.asarray(q_b, f32)),
        "kw": np.ascontiguousarray(np.asarray(k_w, f32)),
        "kb": np.ascontiguousarray(np.asarray(k_b, f32)),
        "kyw": np.ascontiguousarray(np.asarray(ky_w, f32)),
        "kyb": np.ascontiguousarray(np.asarray(ky_b, f32)),
        "cosk": chunk_cs(freqs_cos),
        "sink": chunk_cs(freqs_sin),
        "gates": np.ascontiguousarray(np.tanh(np.asarray(gate, f32))),
    }
    per_core = []
    for c in range(8):
        b, hf = c // 2, c % 2
        sl = slice(hf * S_LOC, (hf + 1) * S_LOC)
        xb = np.asarray(x[b], f32)
        m = dict(shared)
        m["x"] = chunk_x(xb)
        m["xq"] = chunk_x(xb[sl])
        m["y"] = np.ascontiguousarray(
            np.asarray(y[b], f32).T.astype(BF16).reshape(YDC, P, YL)
            .transpose(1, 0, 2))
        m["cosq"] = chunk_cs(np.asarray(freqs_cos, f32)[sl])
        m["sinq"] = chunk_cs(np.asarray(freqs_sin, f32)[sl])
        m["ymb"] = np.where(np.asarray(y_mask[b]), 0.0, -1e9).astype(f32)
        per_core.append(m)
    return per_core


def kernel(**inputs):
    if "nc" not in _CACHED:
        _CACHED["nc"] = build_program()
    nc = _CACHED["nc"]
    in_maps = _prep_inputs(
        inputs["x"], inputs["y"], inputs["freqs_cos"], inputs["freqs_sin"],
        inputs["y_mask"], inputs["wq"], inputs["wk"], inputs["wv"],
        inputs["wk_y"], inputs["wv_y"], inputs["wo"], inputs["q_w"],
        inputs["q_b"], inputs["k_w"], inputs["k_b"], inputs["ky_w"],
        inputs["ky_b"], inputs["gate"])
    res = run_bass_kernel_spmd(nc, in_maps, core_ids=list(range(8)))
    global LAST_EXEC_NS
    LAST_EXEC_NS = res.exec_time_ns
    out = np.zeros((B, S, D), np.float32)
    for c in range(8):
        b, hf = c // 2, c % 2
        out[b, hf * S_LOC:(hf + 1) * S_LOC, :] = res.results[c]["outT"].T
    return out


if __name__ == "__main__":
    nc = build_program()
    print("program built OK")
